# revision 5
# baseline (speedup 1.0000x reference)
"""Multi-head attention (B=4, S=2048, E=1024, H=16, D=64) on 8 trn2 cores.

Sharding: 2D (batch x head-group). Core c handles batch b = c//2 and head
group g = c%2 (8 heads = 512 feature dims). Each core computes a full
[S, E] partial of the output projection for its batch; the host sums the
two group partials per batch and adds the bias.

Per-core device kernel (fp16/bf16 matmuls, fp32 PSUM accumulation):
  qT = (Wq_loc @ X_q^T)      [512, 2048]  (transposed, fp16)
  kT = (Wk_loc @ X_k^T)      [512, 2048]
  v  = X_v @ Wv_loc^T        [2048, 512]  (natural layout + ones column)
  attention processed per head-PAIR (A at partitions 0:64, B at 64:128 of
  the qT/kT chunk), per qq-half (qt), per kk-chunk (128):
    scores^T via two row-tiled matmuls (tile_position (0,0)/(64,0)):
      K=64 each, the two 64-row PE tiles run concurrently -> ~2x scores.
    P_A = exp(scores_A)  on ScalarE (no max subtraction; max |score| ~76
      fits fp32/bf16 range)
    P_B = exp(scores_B)  on VectorE via Schraudolph bf16-bit trick:
      int16(x * 128/ln2 + (127*128 - 7.33)) reinterpreted as bf16.
      ~+-3% elementwise, but softmax normalization cancels the shared
      scale; measured end-to-end rel err ~4e-3 (budget 2e-2).
    U += v_aug^T @ P^T    (row 64 of U = softmax denominator)
  normalize: U copied to SBUF (frees PSUM early), aT = U[0:64] * (1/U[64])
  y = aT^T @ Wo_loc^T   [2048, 1024] f32 partial
"""

from contextlib import ExitStack

import numpy as np

S = 2048
E = 1024
F = 512          # local feature dims (8 heads x 64)
HL = 8           # heads per core
D = 64
B = 4
H = 16
NCORES = 8

# exp(x) ~= bf16_frombits(int16(x * 128/ln2 + B_EXP)); B_EXP centers the
# log-error of the linear-mantissa approximation (127*128 - 7.33, +0.25
# hedge for unknown convert rounding mode).
A_EXP = 184.66496523378731
B_EXP = 16248.92

_CACHE = {}


def build_nc(reps: int = 1):
    import concourse.tile as tile
    from concourse import bacc, mybir

    F16 = mybir.dt.float16
    BF16 = mybir.dt.bfloat16
    F32 = mybir.dt.float32
    I16 = mybir.dt.int16
    EXP = mybir.ActivationFunctionType.Exp
    MULT = mybir.AluOpType.mult
    ADD = mybir.AluOpType.add

    nc = bacc.Bacc(
        "TRN2",
        target_bir_lowering=False,
        debug=False,
        enable_asserts=False,
        num_devices=NCORES,
    )

    xq_d = nc.dram_tensor("xq", [E, S], F16, kind="ExternalInput").ap()
    xk_d = nc.dram_tensor("xk", [E, S], F16, kind="ExternalInput").ap()
    xv_d = nc.dram_tensor("xv", [E, S], F16, kind="ExternalInput").ap()
    wq_d = nc.dram_tensor("wq", [E, F], F16, kind="ExternalInput").ap()
    wk_d = nc.dram_tensor("wk", [E, F], F16, kind="ExternalInput").ap()
    wv_d = nc.dram_tensor("wv", [E, F], F16, kind="ExternalInput").ap()
    wo_d = nc.dram_tensor("wo", [F, E], F16, kind="ExternalInput").ap()
    y_d = nc.dram_tensor("y", [S, E], F32, kind="ExternalOutput").ap()

    with tile.TileContext(nc) as tc, ExitStack() as ctx:
        persist = ctx.enter_context(tc.tile_pool(name="persist", bufs=1))
        xpool = ctx.enter_context(tc.tile_pool(name="xpool", bufs=16))
        ppool = ctx.enter_context(tc.tile_pool(name="ppool", bufs=4))
        stpool = ctx.enter_context(tc.tile_pool(name="stpool", bufs=2))
        ypool = ctx.enter_context(tc.tile_pool(name="ypool", bufs=3))
        ps_s = ctx.enter_context(tc.tile_pool(name="ps_s", bufs=2, space="PSUM"))
        ps_u = ctx.enter_context(tc.tile_pool(name="ps_u", bufs=2, space="PSUM"))

        def body(iv):
            # ---------------- weight/x loads ----------------
            def load_w(dram, pfx, width):
                tiles = []
                nchunks = dram.shape[0] // 128
                for i in range(nchunks):
                    t = persist.tile([128, width], F16, tag=f"{pfx}{i}",
                                     name=f"{pfx}_sb{i}")
                    nc.sync.dma_start(t[:], dram[i * 128:(i + 1) * 128, :])
                    tiles.append(t)
                return tiles

            def load_x(dram, pfx):
                tiles = []
                for eci in range(8):
                    t = xpool.tile([128, S], F16, tag="x", name=f"x{pfx}{eci}")
                    nc.sync.dma_start(t[:], dram[eci * 128:(eci + 1) * 128, :])
                    tiles.append(t)
                return tiles

            # v first (attention depends on all of v); interleave w/x DMAs
            wv_sb, xv_sb = [], []
            for eci in range(8):
                t = persist.tile([128, F], F16, tag=f"wv{eci}",
                                 name=f"wv_sb{eci}")
                nc.sync.dma_start(t[:], wv_d[eci * 128:(eci + 1) * 128, :])
                wv_sb.append(t)
                t = xpool.tile([128, S], F16, tag="x", name=f"xv{eci}")
                nc.sync.dma_start(t[:], xv_d[eci * 128:(eci + 1) * 128, :])
                xv_sb.append(t)

            # v with ones column: v_sb[p, tc, h, d] = v[tc*128+p, h*64+d],
            # d=64 column stays 1.0 (softmax denominator trick)
            v_sb = persist.tile([128, 16, HL, D + 1], BF16, tag="v_sb",
                                name="v_sb")
            nc.vector.memset(v_sb[:], 1.0)

            # ---------------- V projection (natural layout) ----------------
            for tci in range(16):
                vp = ps_u.tile([128, F], F32, tag="u", name=f"v_ps{tci}")
                for eci in range(8):
                    nc.tensor.matmul(
                        vp[:],
                        lhsT=xv_sb[eci][:, tci * 128:(tci + 1) * 128],
                        rhs=wv_sb[eci][:],
                        start=(eci == 0),
                        stop=(eci == 7),
                    )
                # single strided drain (free sizes match: 8*64 == 512);
                # ScalarE is idle this early, keep DVE free for attention
                nc.scalar.copy(v_sb[:, tci, :, 0:D], vp[:])

            # ---------------- Q/K projection chunks (transposed layout) ----
            wq_sb = load_w(wq_d, "wq", F)
            xq_sb = load_x(xq_d, "q")
            wk_sb = load_w(wk_d, "wk", F)
            xk_sb = load_x(xk_d, "k")
            wo_sb = load_w(wo_d, "wo", E)

            qT_sb = [persist.tile([128, S], F16, tag=f"qT{i}", name=f"qT_sb{i}")
                     for i in range(4)]
            kT_sb = [persist.tile([128, S], F16, tag=f"kT{i}", name=f"kT_sb{i}")
                     for i in range(4)]

            def proj_oc(x_sb, w_sb, ot, oci, pfx):
                for half in range(2):
                    pp = ps_s.tile([128, 1024], F32, tag="s",
                                   name=f"{pfx}p{oci}_{half}")
                    for eci in range(8):
                        for nb in range(2):
                            col = half * 1024 + nb * 512
                            nc.tensor.matmul(
                                pp[:, nb * 512:(nb + 1) * 512],
                                lhsT=w_sb[eci][:, oci * 128:(oci + 1) * 128],
                                rhs=x_sb[eci][:, col:col + 512],
                                start=(eci == 0),
                                stop=(eci == 7),
                            )
                    nc.scalar.copy(ot[:, half * 1024:(half + 1) * 1024], pp[:])

            # attnout^T storage
            aT_sb = [persist.tile([128, S], F16, tag=f"aT{i}", name=f"aT_sb{i}")
                     for i in range(4)]

            # ---------------- attention for one head pair ----------------
            def attn_pair(ch):
                hA, hB = 2 * ch, 2 * ch + 1

                def av(U, hh, kk, pt):
                    for nb in range(2):
                        nc.tensor.matmul(
                            U[:, nb * 512:(nb + 1) * 512],
                            lhsT=v_sb[:, kk, hh, :],
                            rhs=pt[:, nb * 512:(nb + 1) * 512],
                            start=(kk == 0),
                            stop=(kk == 15),
                        )

                for qt in range(2):
                    U_A = ps_u.tile([65, 1024], F32, tag="u",
                                    name=f"UA{ch}_{qt}")
                    U_B = ps_u.tile([65, 1024], F32, tag="u",
                                    name=f"UB{ch}_{qt}")
                    prevA = prevB = None
                    for kk in range(16):
                        scA = ps_s.tile([128, 1024], F32, tag="s",
                                        name=f"scA{ch}_{qt}_{kk}")
                        scB = ps_s.tile([128, 1024], F32, tag="s",
                                        name=f"scB{ch}_{qt}_{kk}")
                        # row-tiled pair: K=64 each, concurrent PE tiles
                        for nb in range(2):
                            qcol = qt * 1024 + nb * 512
                            nc.tensor.matmul(
                                scA[:, nb * 512:(nb + 1) * 512],
                                lhsT=kT_sb[ch][0:64, kk * 128:(kk + 1) * 128],
                                rhs=qT_sb[ch][0:64, qcol:qcol + 512],
                                start=True, stop=True,
                                tile_position=(0, 0),
                            )
                            nc.tensor.matmul(
                                scB[:, nb * 512:(nb + 1) * 512],
                                lhsT=kT_sb[ch][64:128, kk * 128:(kk + 1) * 128],
                                rhs=qT_sb[ch][64:128, qcol:qcol + 512],
                                start=True, stop=True,
                                tile_position=(64, 0),
                            )
                        # AV of previous chunk keeps the PE fed during exp
                        if prevA is not None:
                            av(U_A, hA, kk - 1, prevA)
                            av(U_B, hB, kk - 1, prevB)
                        ptA = ppool.tile([128, 1024], BF16, tag="p",
                                         name=f"pA{ch}_{qt}_{kk}")
                        nc.scalar.activation(ptA[:], scA[:], EXP)
                        ptB = ppool.tile([128, 1024], I16, tag="p",
                                         name=f"pB{ch}_{qt}_{kk}")
                        nc.vector.tensor_scalar(ptB[:], scB[:], A_EXP, B_EXP,
                                                MULT, ADD)
                        prevA = ptA[:]
                        prevB = ptB[:].bitcast(BF16)
                    av(U_A, hA, 15, prevA)
                    av(U_B, hB, 15, prevB)

                    # normalize via bf16 SBUF staging (frees U's PSUM
                    # quickly, bf16 gets 2x DVE modes): aT = U[0:64] / U[64]
                    for U, p0, hh in ((U_A, 0, hA), (U_B, 64, hB)):
                        st = stpool.tile([65, 1024], BF16, tag="st",
                                         name=f"st{hh}_{qt}")
                        nc.scalar.copy(st[:], U[:])
                        rcp = stpool.tile([1, 1024], BF16, tag="rcp",
                                          name=f"rcp{hh}_{qt}")
                        with nc.allow_low_precision(reason="softmax denom"):
                            nc.vector.reciprocal(rcp[:], st[64:65, :])
                        bc = stpool.tile([64, 1024], BF16, tag="bc",
                                         name=f"bc{hh}_{qt}")
                        nc.gpsimd.partition_broadcast(bc[:], rcp[:])
                        nc.vector.tensor_mul(
                            aT_sb[ch][p0:p0 + 64, qt * 1024:(qt + 1) * 1024],
                            st[0:64, :], bc[:])

            # first q/k chunk upfront, later chunks between pairs
            proj_oc(xq_sb, wq_sb, qT_sb[0], 0, "q")
            proj_oc(xk_sb, wk_sb, kT_sb[0], 0, "k")
            attn_pair(0)
            proj_oc(xq_sb, wq_sb, qT_sb[1], 1, "q")
            proj_oc(xk_sb, wk_sb, kT_sb[1], 1, "k")
            attn_pair(1)
            proj_oc(xq_sb, wq_sb, qT_sb[2], 2, "q")
            proj_oc(xk_sb, wk_sb, kT_sb[2], 2, "k")
            attn_pair(2)
            proj_oc(xq_sb, wq_sb, qT_sb[3], 3, "q")
            proj_oc(xk_sb, wk_sb, kT_sb[3], 3, "k")
            attn_pair(3)

            # ---------------- output projection ----------------
            for tci in range(16):
                yp = ps_u.tile([128, 1024], F32, tag="u", name=f"y_ps{tci}")
                for fc in range(4):
                    for nb in range(2):
                        nc.tensor.matmul(
                            yp[:, nb * 512:(nb + 1) * 512],
                            lhsT=aT_sb[fc][:, tci * 128:(tci + 1) * 128],
                            rhs=wo_sb[fc][:, nb * 512:(nb + 1) * 512],
                            start=(fc == 0),
                            stop=(fc == 3),
                        )
                ysb = ypool.tile([128, 1024], F32, tag="y", name=f"y_sb{tci}")
                # split PSUM drain between ACT and DVE (both idle at tail)
                if tci % 2 == 0:
                    nc.scalar.copy(ysb[:], yp[:])
                else:
                    nc.vector.tensor_copy(ysb[:], yp[:])
                nc.sync.dma_start(y_d[tci * 128:(tci + 1) * 128, :], ysb[:])

        if reps == 1:
            body(0)
        else:
            with tc.For_i(0, reps, 1) as iv:
                body(iv)

    nc.compile()
    return nc


def make_in_maps(Q, K, V, Wq, Wk, Wv, Wo):
    """Shard + lay out full inputs for the 8 cores."""
    Q = np.asarray(Q, dtype=np.float32)
    K = np.asarray(K, dtype=np.float32)
    V = np.asarray(V, dtype=np.float32)
    Wq = np.asarray(Wq, dtype=np.float32)
    Wk = np.asarray(Wk, dtype=np.float32)
    Wv = np.asarray(Wv, dtype=np.float32)
    Wo = np.asarray(Wo, dtype=np.float32)

    in_maps = []
    for c in range(NCORES):
        b, g = c // 2, c % 2
        rows = slice(g * F, (g + 1) * F)
        in_maps.append({
            "xq": np.ascontiguousarray(Q[b].T).astype(np.float16),
            "xk": np.ascontiguousarray(K[b].T).astype(np.float16),
            "xv": np.ascontiguousarray(V[b].T).astype(np.float16),
            "wq": np.ascontiguousarray(Wq[rows, :].T).astype(np.float16),
            "wk": np.ascontiguousarray(Wk[rows, :].T).astype(np.float16),
            "wv": np.ascontiguousarray(Wv[rows, :].T).astype(np.float16),
            "wo": np.ascontiguousarray(Wo[:, rows].T).astype(np.float16),
        })
    return in_maps


def combine(results, bo):
    """Sum per-core partials + bias -> full [B, S, E] output."""
    bo = np.asarray(bo, dtype=np.float32)
    y = np.zeros((B, S, E), dtype=np.float32)
    for c in range(NCORES):
        y[c // 2] += results[c]["y"]
    y += bo[None, None, :]
    return y


def kernel(Q, K, V, Wq, Wk, Wv, Wo, bo):
    from concourse.bass_utils import run_bass_kernel_spmd

    if "nc" not in _CACHE:
        _CACHE["nc"] = build_nc(reps=1)
    nc = _CACHE["nc"]
    in_maps = make_in_maps(Q, K, V, Wq, Wk, Wv, Wo)
    res = run_bass_kernel_spmd(nc, in_maps, core_ids=list(range(NCORES)))
    return combine(res.results, bo)


# revision 6
# speedup vs baseline: 1.1681x; 1.1681x over previous
"""Multi-head attention (B=4, S=2048, E=1024, H=16, D=64) on 8 trn2 cores.

Sharding: 2D (batch x head-group). Core c handles batch b = c//2 and head
group g = c%2 (8 heads = 512 feature dims). Each core computes a full
[S, E] partial of the output projection for its batch; the host sums the
two group partials per batch and adds the bias.

Per-core device kernel (fp16/bf16 matmuls, fp32 PSUM accumulation):
  qT = (Wq_loc @ X_q^T)      [512, 2048]  (transposed, fp16)
  kT = (Wk_loc @ X_k^T)      [512, 2048]
  v  = X_v @ Wv_loc^T        [2048, 512]  (natural layout + ones column)
  attention processed per head-PAIR (A at partitions 0:64, B at 64:128 of
  the qT/kT chunk), per qq-half (qt), per kk-chunk (128):
    scores^T via two row-tiled matmuls (tile_position (0,0)/(64,0)):
      K=64 each, the two 64-row PE tiles run concurrently -> ~2x scores.
    P_A = exp(scores_A)  on ScalarE (no max subtraction; max |score| ~76
      fits fp32/bf16 range)
    P_B = exp(scores_B)  on VectorE via Schraudolph bf16-bit trick:
      int16(x * 128/ln2 + (127*128 - 7.33)) reinterpreted as bf16.
      ~+-3% elementwise, but softmax normalization cancels the shared
      scale; measured end-to-end rel err ~4e-3 (budget 2e-2).
    U += v_aug^T @ P^T    (row 64 of U = softmax denominator)
  normalize: U copied to SBUF (frees PSUM early), aT = U[0:64] * (1/U[64])
  y = aT^T @ Wo_loc^T   [2048, 1024] f32 partial
"""

from contextlib import ExitStack

import numpy as np

S = 2048
E = 1024
F = 512          # local feature dims (8 heads x 64)
HL = 8           # heads per core
D = 64
B = 4
H = 16
NCORES = 8

# exp(x) ~= bf16_frombits(int16(x * 128/ln2 + B_EXP)); B_EXP centers the
# log-error of the linear-mantissa approximation (127*128 - 7.33, +0.25
# hedge for unknown convert rounding mode).
A_EXP = 184.66496523378731
B_EXP = 16248.92

_CACHE = {}


def build_nc(reps: int = 1):
    import concourse.tile as tile
    from concourse import bacc, mybir

    F16 = mybir.dt.float16
    BF16 = mybir.dt.bfloat16
    F32 = mybir.dt.float32
    I16 = mybir.dt.int16
    EXP = mybir.ActivationFunctionType.Exp
    MULT = mybir.AluOpType.mult
    ADD = mybir.AluOpType.add

    nc = bacc.Bacc(
        "TRN2",
        target_bir_lowering=False,
        debug=False,
        enable_asserts=False,
        num_devices=NCORES,
    )

    xq_d = nc.dram_tensor("xq", [E, S], F16, kind="ExternalInput").ap()
    xk_d = nc.dram_tensor("xk", [E, S], F16, kind="ExternalInput").ap()
    xv_d = nc.dram_tensor("xv", [E, S], F16, kind="ExternalInput").ap()
    wq_d = nc.dram_tensor("wq", [E, F], F16, kind="ExternalInput").ap()
    wk_d = nc.dram_tensor("wk", [E, F], F16, kind="ExternalInput").ap()
    wv_d = nc.dram_tensor("wv", [E, F], F16, kind="ExternalInput").ap()
    wo_d = nc.dram_tensor("wo", [F, E], F16, kind="ExternalInput").ap()
    y_d = nc.dram_tensor("y", [S, E], F32, kind="ExternalOutput").ap()

    with tile.TileContext(nc) as tc, ExitStack() as ctx:
        persist = ctx.enter_context(tc.tile_pool(name="persist", bufs=1))
        xpool = ctx.enter_context(tc.tile_pool(name="xpool", bufs=16))
        ppool = ctx.enter_context(tc.tile_pool(name="ppool", bufs=4))
        stpool = ctx.enter_context(tc.tile_pool(name="stpool", bufs=2))
        ypool = ctx.enter_context(tc.tile_pool(name="ypool", bufs=3))
        ps_s = ctx.enter_context(tc.tile_pool(name="ps_s", bufs=2, space="PSUM"))
        ps_u = ctx.enter_context(tc.tile_pool(name="ps_u", bufs=2, space="PSUM"))

        def body(iv):
            # ---------------- weight/x loads ----------------
            def load_w(dram, pfx, width):
                tiles = []
                nchunks = dram.shape[0] // 128
                for i in range(nchunks):
                    t = persist.tile([128, width], F16, tag=f"{pfx}{i}",
                                     name=f"{pfx}_sb{i}")
                    nc.sync.dma_start(t[:], dram[i * 128:(i + 1) * 128, :])
                    tiles.append(t)
                return tiles

            def load_x(dram, pfx):
                tiles = []
                for eci in range(8):
                    t = xpool.tile([128, S], F16, tag="x", name=f"x{pfx}{eci}")
                    nc.sync.dma_start(t[:], dram[eci * 128:(eci + 1) * 128, :])
                    tiles.append(t)
                return tiles

            # v first (attention depends on all of v); interleave w/x DMAs
            wv_sb, xv_sb = [], []
            for eci in range(8):
                t = persist.tile([128, F], F16, tag=f"wv{eci}",
                                 name=f"wv_sb{eci}")
                nc.sync.dma_start(t[:], wv_d[eci * 128:(eci + 1) * 128, :])
                wv_sb.append(t)
                t = xpool.tile([128, S], F16, tag="x", name=f"xv{eci}")
                nc.sync.dma_start(t[:], xv_d[eci * 128:(eci + 1) * 128, :])
                xv_sb.append(t)

            # v with ones column: v_sb[p, tc, h, d] = v[tc*128+p, h*64+d],
            # d=64 column stays 1.0 (softmax denominator trick)
            v_sb = persist.tile([128, 16, HL, D + 1], BF16, tag="v_sb",
                                name="v_sb")
            nc.vector.memset(v_sb[:], 1.0)

            # ---------------- V projection (natural layout) ----------------
            for tci in range(16):
                vp = ps_u.tile([128, F], F32, tag="u", name=f"v_ps{tci}")
                for eci in range(8):
                    nc.tensor.matmul(
                        vp[:],
                        lhsT=xv_sb[eci][:, tci * 128:(tci + 1) * 128],
                        rhs=wv_sb[eci][:],
                        start=(eci == 0),
                        stop=(eci == 7),
                    )
                # single strided drain (free sizes match: 8*64 == 512);
                # ScalarE is idle this early, keep DVE free for attention
                nc.scalar.copy(v_sb[:, tci, :, 0:D], vp[:])

            # ---------------- Q/K projection chunks (transposed layout) ----
            wq_sb = load_w(wq_d, "wq", F)
            xq_sb = load_x(xq_d, "q")
            wk_sb = load_w(wk_d, "wk", F)
            xk_sb = load_x(xk_d, "k")
            wo_sb = load_w(wo_d, "wo", E)

            qT_sb = [persist.tile([128, S], F16, tag=f"qT{i}", name=f"qT_sb{i}")
                     for i in range(4)]
            kT_sb = [persist.tile([128, S], F16, tag=f"kT{i}", name=f"kT_sb{i}")
                     for i in range(4)]

            def proj_oc(x_sb, w_sb, ot, oci, pfx):
                for half in range(2):
                    pp = ps_s.tile([128, 1024], F32, tag="s",
                                   name=f"{pfx}p{oci}_{half}")
                    for eci in range(8):
                        for nb in range(2):
                            col = half * 1024 + nb * 512
                            nc.tensor.matmul(
                                pp[:, nb * 512:(nb + 1) * 512],
                                lhsT=w_sb[eci][:, oci * 128:(oci + 1) * 128],
                                rhs=x_sb[eci][:, col:col + 512],
                                start=(eci == 0),
                                stop=(eci == 7),
                            )
                    nc.scalar.copy(ot[:, half * 1024:(half + 1) * 1024], pp[:])

            # attnout^T storage
            aT_sb = [persist.tile([128, S], F16, tag=f"aT{i}", name=f"aT_sb{i}")
                     for i in range(4)]

            # ---------------- attention for one head pair ----------------
            def attn_pair(ch):
                hA, hB = 2 * ch, 2 * ch + 1

                def av(U, hh, kk, pt):
                    for nb in range(2):
                        nc.tensor.matmul(
                            U[:, nb * 512:(nb + 1) * 512],
                            lhsT=v_sb[:, kk, hh, :],
                            rhs=pt[:, nb * 512:(nb + 1) * 512],
                            start=(kk == 0),
                            stop=(kk == 15),
                        )

                for qt in range(2):
                    U_A = ps_u.tile([65, 1024], F32, tag="u",
                                    name=f"UA{ch}_{qt}")
                    U_B = ps_u.tile([65, 1024], F32, tag="u",
                                    name=f"UB{ch}_{qt}")
                    prevA = prevB = None
                    for kk in range(16):
                        scA = ps_s.tile([128, 1024], F32, tag="s",
                                        name=f"scA{ch}_{qt}_{kk}")
                        scB = ps_s.tile([128, 1024], F32, tag="s",
                                        name=f"scB{ch}_{qt}_{kk}")
                        # K=64 per head; plain matmuls (tile_position row
                        # packing measured SLOWER on HW here: the per-kk
                        # 64-row <-> 128x128 mode switches drain the PE)
                        for nb in range(2):
                            qcol = qt * 1024 + nb * 512
                            nc.tensor.matmul(
                                scA[:, nb * 512:(nb + 1) * 512],
                                lhsT=kT_sb[ch][0:64, kk * 128:(kk + 1) * 128],
                                rhs=qT_sb[ch][0:64, qcol:qcol + 512],
                                start=True, stop=True,
                            )
                            nc.tensor.matmul(
                                scB[:, nb * 512:(nb + 1) * 512],
                                lhsT=kT_sb[ch][64:128, kk * 128:(kk + 1) * 128],
                                rhs=qT_sb[ch][64:128, qcol:qcol + 512],
                                start=True, stop=True,
                            )
                        # AV of previous chunk keeps the PE fed during exp
                        if prevA is not None:
                            av(U_A, hA, kk - 1, prevA)
                            av(U_B, hB, kk - 1, prevB)
                        ptA = ppool.tile([128, 1024], BF16, tag="p",
                                         name=f"pA{ch}_{qt}_{kk}")
                        nc.scalar.activation(ptA[:], scA[:], EXP)
                        ptB = ppool.tile([128, 1024], I16, tag="p",
                                         name=f"pB{ch}_{qt}_{kk}")
                        nc.vector.tensor_scalar(ptB[:], scB[:], A_EXP, B_EXP,
                                                MULT, ADD)
                        prevA = ptA[:]
                        prevB = ptB[:].bitcast(BF16)
                    av(U_A, hA, 15, prevA)
                    av(U_B, hB, 15, prevB)

                    # normalize via bf16 SBUF staging (frees U's PSUM
                    # quickly, bf16 gets 2x DVE modes): aT = U[0:64] / U[64]
                    for U, p0, hh in ((U_A, 0, hA), (U_B, 64, hB)):
                        st = stpool.tile([65, 1024], BF16, tag="st",
                                         name=f"st{hh}_{qt}")
                        nc.scalar.copy(st[:], U[:])
                        rcp = stpool.tile([1, 1024], BF16, tag="rcp",
                                          name=f"rcp{hh}_{qt}")
                        with nc.allow_low_precision(reason="softmax denom"):
                            nc.vector.reciprocal(rcp[:], st[64:65, :])
                        bc = stpool.tile([64, 1024], BF16, tag="bc",
                                         name=f"bc{hh}_{qt}")
                        nc.gpsimd.partition_broadcast(bc[:], rcp[:])
                        nc.vector.tensor_mul(
                            aT_sb[ch][p0:p0 + 64, qt * 1024:(qt + 1) * 1024],
                            st[0:64, :], bc[:])

            # first q/k chunk upfront, later chunks between pairs
            proj_oc(xq_sb, wq_sb, qT_sb[0], 0, "q")
            proj_oc(xk_sb, wk_sb, kT_sb[0], 0, "k")
            attn_pair(0)
            proj_oc(xq_sb, wq_sb, qT_sb[1], 1, "q")
            proj_oc(xk_sb, wk_sb, kT_sb[1], 1, "k")
            attn_pair(1)
            proj_oc(xq_sb, wq_sb, qT_sb[2], 2, "q")
            proj_oc(xk_sb, wk_sb, kT_sb[2], 2, "k")
            attn_pair(2)
            proj_oc(xq_sb, wq_sb, qT_sb[3], 3, "q")
            proj_oc(xk_sb, wk_sb, kT_sb[3], 3, "k")
            attn_pair(3)

            # ---------------- output projection ----------------
            for tci in range(16):
                yp = ps_u.tile([128, 1024], F32, tag="u", name=f"y_ps{tci}")
                for fc in range(4):
                    for nb in range(2):
                        nc.tensor.matmul(
                            yp[:, nb * 512:(nb + 1) * 512],
                            lhsT=aT_sb[fc][:, tci * 128:(tci + 1) * 128],
                            rhs=wo_sb[fc][:, nb * 512:(nb + 1) * 512],
                            start=(fc == 0),
                            stop=(fc == 3),
                        )
                ysb = ypool.tile([128, 1024], F32, tag="y", name=f"y_sb{tci}")
                # split PSUM drain between ACT and DVE (both idle at tail)
                if tci % 2 == 0:
                    nc.scalar.copy(ysb[:], yp[:])
                else:
                    nc.vector.tensor_copy(ysb[:], yp[:])
                nc.sync.dma_start(y_d[tci * 128:(tci + 1) * 128, :], ysb[:])

        if reps == 1:
            body(0)
        else:
            with tc.For_i(0, reps, 1) as iv:
                body(iv)

    nc.compile()
    return nc


def make_in_maps(Q, K, V, Wq, Wk, Wv, Wo):
    """Shard + lay out full inputs for the 8 cores."""
    Q = np.asarray(Q, dtype=np.float32)
    K = np.asarray(K, dtype=np.float32)
    V = np.asarray(V, dtype=np.float32)
    Wq = np.asarray(Wq, dtype=np.float32)
    Wk = np.asarray(Wk, dtype=np.float32)
    Wv = np.asarray(Wv, dtype=np.float32)
    Wo = np.asarray(Wo, dtype=np.float32)

    in_maps = []
    for c in range(NCORES):
        b, g = c // 2, c % 2
        rows = slice(g * F, (g + 1) * F)
        in_maps.append({
            "xq": np.ascontiguousarray(Q[b].T).astype(np.float16),
            "xk": np.ascontiguousarray(K[b].T).astype(np.float16),
            "xv": np.ascontiguousarray(V[b].T).astype(np.float16),
            "wq": np.ascontiguousarray(Wq[rows, :].T).astype(np.float16),
            "wk": np.ascontiguousarray(Wk[rows, :].T).astype(np.float16),
            "wv": np.ascontiguousarray(Wv[rows, :].T).astype(np.float16),
            "wo": np.ascontiguousarray(Wo[:, rows].T).astype(np.float16),
        })
    return in_maps


def combine(results, bo):
    """Sum per-core partials + bias -> full [B, S, E] output."""
    bo = np.asarray(bo, dtype=np.float32)
    y = np.zeros((B, S, E), dtype=np.float32)
    for c in range(NCORES):
        y[c // 2] += results[c]["y"]
    y += bo[None, None, :]
    return y


def kernel(Q, K, V, Wq, Wk, Wv, Wo, bo):
    from concourse.bass_utils import run_bass_kernel_spmd

    if "nc" not in _CACHE:
        _CACHE["nc"] = build_nc(reps=1)
    nc = _CACHE["nc"]
    in_maps = make_in_maps(Q, K, V, Wq, Wk, Wv, Wo)
    res = run_bass_kernel_spmd(nc, in_maps, core_ids=list(range(NCORES)))
    return combine(res.results, bo)


# revision 7
# speedup vs baseline: 1.3620x; 1.1661x over previous
"""Multi-head attention (B=4, S=2048, E=1024, H=16, D=64) on 8 trn2 cores.

Sharding: 2D (batch x head-group). Core c handles batch b = c//2 and head
group g = c%2 (8 heads = 512 feature dims). Each core computes a full
[S, E] partial of the output projection for its batch; the host sums the
two group partials per batch and adds the bias.

Per-core device kernel (fp16/bf16 matmuls, fp32 PSUM accumulation):
  qT = (A_EXP * Wq_loc @ X_q^T)  [512, 2048]  (transposed, fp16; the
       A_EXP=128/ln2 pre-scale makes both exp paths below one op)
  kT = (Wk_loc @ X_k^T)          [512, 2048]
  v  = X_v @ Wv_loc^T            [2048, 512]  (natural + ones column)
  attention per head-PAIR (A: partitions 0:64, B: 64:128 of the qT/kT
  chunk), per 512-wide qq window, per kk-chunk (128):
    sc[128,1024] = [scores_A | scores_B]   (two K=64 matmuls)
    pt = exp(sc/A_EXP), one merged [128,1024] op alternating per kk:
      even kk: ScalarE activation Exp with scale=1/A_EXP (exact)
      odd  kk: VectorE int16(sc + B_EXP) bitcast bf16 (Schraudolph 2^x
        bit trick, ~+-3% elementwise; softmax normalization cancels the
        shared scale -- measured end-to-end rel err ~5e-3 vs 2e-2 budget)
    U_A += v_aug_A^T @ pt[:,0:512];  U_B += v_aug_B^T @ pt[:,512:1024]
    (row 64 of U = softmax denominator via the v ones column)
  normalize: U staged to SBUF bf16 (frees PSUM), aT = U[0:64] * 1/U[64]
  y = aT^T @ Wo_loc^T  [2048, 1024] f32 partial
"""

from contextlib import ExitStack

import numpy as np

S = 2048
E = 1024
F = 512          # local feature dims (8 heads x 64)
HL = 8           # heads per core
D = 64
B = 4
H = 16
NCORES = 8

# exp(x) ~= bf16_frombits(int16(x*128/ln2 + B_EXP)); wq is pre-scaled by
# A_EXP on the host so the device-side op is a single add.
A_EXP = 184.66496523378731
B_EXP = 16248.92

_CACHE = {}


def build_nc(reps: int = 1):
    import concourse.tile as tile
    from concourse import bacc, mybir

    F16 = mybir.dt.float16
    BF16 = mybir.dt.bfloat16
    F32 = mybir.dt.float32
    I16 = mybir.dt.int16
    EXP = mybir.ActivationFunctionType.Exp

    nc = bacc.Bacc(
        "TRN2",
        target_bir_lowering=False,
        debug=False,
        enable_asserts=False,
        num_devices=NCORES,
    )

    xq_d = nc.dram_tensor("xq", [E, S], F16, kind="ExternalInput").ap()
    xk_d = nc.dram_tensor("xk", [E, S], F16, kind="ExternalInput").ap()
    xv_d = nc.dram_tensor("xv", [E, S], F16, kind="ExternalInput").ap()
    wq_d = nc.dram_tensor("wq", [E, F], F16, kind="ExternalInput").ap()
    wk_d = nc.dram_tensor("wk", [E, F], F16, kind="ExternalInput").ap()
    wv_d = nc.dram_tensor("wv", [E, F], F16, kind="ExternalInput").ap()
    wo_d = nc.dram_tensor("wo", [F, E], F16, kind="ExternalInput").ap()
    y_d = nc.dram_tensor("y", [S, E], F32, kind="ExternalOutput").ap()

    with tile.TileContext(nc) as tc, ExitStack() as ctx:
        persist = ctx.enter_context(tc.tile_pool(name="persist", bufs=1))
        xpool = ctx.enter_context(tc.tile_pool(name="xpool", bufs=16))
        ppool = ctx.enter_context(tc.tile_pool(name="ppool", bufs=4))
        stpool = ctx.enter_context(tc.tile_pool(name="stpool", bufs=2))
        ypool = ctx.enter_context(tc.tile_pool(name="ypool", bufs=3))
        ps_s = ctx.enter_context(tc.tile_pool(name="ps_s", bufs=3, space="PSUM"))
        ps_u = ctx.enter_context(tc.tile_pool(name="ps_u", bufs=2, space="PSUM"))

        def body(iv):
            # ---------------- weight/x loads ----------------
            def load_w(dram, pfx, width):
                tiles = []
                nchunks = dram.shape[0] // 128
                for i in range(nchunks):
                    t = persist.tile([128, width], F16, tag=f"{pfx}{i}",
                                     name=f"{pfx}_sb{i}")
                    nc.sync.dma_start(t[:], dram[i * 128:(i + 1) * 128, :])
                    tiles.append(t)
                return tiles

            def load_x(dram, pfx):
                tiles = []
                for eci in range(8):
                    t = xpool.tile([128, S], F16, tag="x", name=f"x{pfx}{eci}")
                    nc.sync.dma_start(t[:], dram[eci * 128:(eci + 1) * 128, :])
                    tiles.append(t)
                return tiles

            # v first (attention depends on all of v); interleave w/x DMAs
            wv_sb, xv_sb = [], []
            for eci in range(8):
                t = persist.tile([128, F], F16, tag=f"wv{eci}",
                                 name=f"wv_sb{eci}")
                nc.sync.dma_start(t[:], wv_d[eci * 128:(eci + 1) * 128, :])
                wv_sb.append(t)
                t = xpool.tile([128, S], F16, tag="x", name=f"xv{eci}")
                nc.sync.dma_start(t[:], xv_d[eci * 128:(eci + 1) * 128, :])
                xv_sb.append(t)

            # v with ones column: v_sb[p, tc, h, d] = v[tc*128+p, h*64+d],
            # d=64 column stays 1.0 (softmax denominator trick)
            v_sb = persist.tile([128, 16, HL, D + 1], BF16, tag="v_sb",
                                name="v_sb")
            nc.vector.memset(v_sb[:], 1.0)

            # ---------------- V projection (natural layout) ----------------
            for tci in range(16):
                vp = ps_u.tile([128, F], F32, tag="u", name=f"v_ps{tci}")
                for eci in range(8):
                    nc.tensor.matmul(
                        vp[:],
                        lhsT=xv_sb[eci][:, tci * 128:(tci + 1) * 128],
                        rhs=wv_sb[eci][:],
                        start=(eci == 0),
                        stop=(eci == 7),
                    )
                for h in range(HL):
                    nc.vector.tensor_copy(v_sb[:, tci, h, 0:D],
                                          vp[:, h * D:(h + 1) * D])

            # ---------------- Q/K projection chunks (transposed layout) ----
            wq_sb = load_w(wq_d, "wq", F)
            xq_sb = load_x(xq_d, "q")
            wk_sb = load_w(wk_d, "wk", F)
            xk_sb = load_x(xk_d, "k")
            wo_sb = load_w(wo_d, "wo", E)

            qT_sb = [persist.tile([128, S], F16, tag=f"qT{i}", name=f"qT_sb{i}")
                     for i in range(4)]
            kT_sb = [persist.tile([128, S], F16, tag=f"kT{i}", name=f"kT_sb{i}")
                     for i in range(4)]

            def proj_oc(x_sb, w_sb, ot, oci, pfx):
                for half in range(2):
                    pp = ps_s.tile([128, 1024], F32, tag="s",
                                   name=f"{pfx}p{oci}_{half}")
                    for eci in range(8):
                        for nb in range(2):
                            col = half * 1024 + nb * 512
                            nc.tensor.matmul(
                                pp[:, nb * 512:(nb + 1) * 512],
                                lhsT=w_sb[eci][:, oci * 128:(oci + 1) * 128],
                                rhs=x_sb[eci][:, col:col + 512],
                                start=(eci == 0),
                                stop=(eci == 7),
                            )
                    nc.vector.tensor_copy(
                        ot[:, half * 1024:(half + 1) * 1024], pp[:])

            # attnout^T storage
            aT_sb = [persist.tile([128, S], F16, tag=f"aT{i}", name=f"aT_sb{i}")
                     for i in range(4)]

            # ---------------- attention for one head pair ----------------
            def attn_pair(ch):
                hA, hB = 2 * ch, 2 * ch + 1
                for qw in range(4):
                    qcol = qw * 512
                    U_A = ps_u.tile([65, 512], F32, tag="u",
                                    name=f"UA{ch}_{qw}")
                    U_B = ps_u.tile([65, 512], F32, tag="u",
                                    name=f"UB{ch}_{qw}")
                    prev = None
                    for kk in range(16):
                        sc = ps_s.tile([128, 1024], F32, tag="s",
                                       name=f"sc{ch}_{qw}_{kk}")
                        nc.tensor.matmul(
                            sc[:, 0:512],
                            lhsT=kT_sb[ch][0:64, kk * 128:(kk + 1) * 128],
                            rhs=qT_sb[ch][0:64, qcol:qcol + 512],
                            start=True, stop=True,
                        )
                        nc.tensor.matmul(
                            sc[:, 512:1024],
                            lhsT=kT_sb[ch][64:128, kk * 128:(kk + 1) * 128],
                            rhs=qT_sb[ch][64:128, qcol:qcol + 512],
                            start=True, stop=True,
                        )
                        if prev is not None:
                            nc.tensor.matmul(
                                U_A[:], lhsT=v_sb[:, kk - 1, hA, :],
                                rhs=prev[:, 0:512],
                                start=(kk == 1), stop=False)
                            nc.tensor.matmul(
                                U_B[:], lhsT=v_sb[:, kk - 1, hB, :],
                                rhs=prev[:, 512:1024],
                                start=(kk == 1), stop=False)
                        # merged exp over both heads, alternating engines
                        if kk % 2 == 0:
                            pt = ppool.tile([128, 1024], BF16, tag="p",
                                            name=f"p{ch}_{qw}_{kk}")
                            nc.scalar.activation(pt[:], sc[:], EXP,
                                                 scale=1.0 / A_EXP)
                            prev = pt[:]
                        else:
                            pt = ppool.tile([128, 1024], I16, tag="p",
                                            name=f"p{ch}_{qw}_{kk}")
                            nc.vector.tensor_scalar_add(pt[:], sc[:], B_EXP)
                            prev = pt[:].bitcast(BF16)
                    nc.tensor.matmul(U_A[:], lhsT=v_sb[:, 15, hA, :],
                                     rhs=prev[:, 0:512],
                                     start=False, stop=True)
                    nc.tensor.matmul(U_B[:], lhsT=v_sb[:, 15, hB, :],
                                     rhs=prev[:, 512:1024],
                                     start=False, stop=True)

                    # normalize via bf16 SBUF staging (frees U quickly):
                    # aT = U[0:64] / U[64]
                    for U, p0, hh in ((U_A, 0, hA), (U_B, 64, hB)):
                        st = stpool.tile([65, 512], BF16, tag="st",
                                         name=f"st{hh}_{qw}")
                        nc.scalar.copy(st[:], U[:])
                        rcp = stpool.tile([1, 512], BF16, tag="rcp",
                                          name=f"rcp{hh}_{qw}")
                        with nc.allow_low_precision(reason="softmax denom"):
                            nc.vector.reciprocal(rcp[:], st[64:65, :])
                        bc = stpool.tile([64, 512], BF16, tag="bc",
                                         name=f"bc{hh}_{qw}")
                        nc.gpsimd.partition_broadcast(bc[:], rcp[:])
                        nc.vector.tensor_mul(
                            aT_sb[ch][p0:p0 + 64, qcol:qcol + 512],
                            st[0:64, :], bc[:])

            # first q/k chunk upfront, later chunks between pairs
            proj_oc(xq_sb, wq_sb, qT_sb[0], 0, "q")
            proj_oc(xk_sb, wk_sb, kT_sb[0], 0, "k")
            attn_pair(0)
            proj_oc(xq_sb, wq_sb, qT_sb[1], 1, "q")
            proj_oc(xk_sb, wk_sb, kT_sb[1], 1, "k")
            attn_pair(1)
            proj_oc(xq_sb, wq_sb, qT_sb[2], 2, "q")
            proj_oc(xk_sb, wk_sb, kT_sb[2], 2, "k")
            attn_pair(2)
            proj_oc(xq_sb, wq_sb, qT_sb[3], 3, "q")
            proj_oc(xk_sb, wk_sb, kT_sb[3], 3, "k")
            attn_pair(3)

            # ---------------- output projection ----------------
            for tci in range(16):
                yp = ps_s.tile([128, 1024], F32, tag="s", name=f"y_ps{tci}")
                for fc in range(4):
                    for nb in range(2):
                        nc.tensor.matmul(
                            yp[:, nb * 512:(nb + 1) * 512],
                            lhsT=aT_sb[fc][:, tci * 128:(tci + 1) * 128],
                            rhs=wo_sb[fc][:, nb * 512:(nb + 1) * 512],
                            start=(fc == 0),
                            stop=(fc == 3),
                        )
                ysb = ypool.tile([128, 1024], F32, tag="y", name=f"y_sb{tci}")
                # split PSUM drain between ACT and DVE (both idle at tail)
                if tci % 2 == 0:
                    nc.scalar.copy(ysb[:], yp[:])
                else:
                    nc.vector.tensor_copy(ysb[:], yp[:])
                nc.sync.dma_start(y_d[tci * 128:(tci + 1) * 128, :], ysb[:])

        if reps == 1:
            body(0)
        else:
            with tc.For_i(0, reps, 1) as iv:
                body(iv)

    nc.compile()
    return nc


def make_in_maps(Q, K, V, Wq, Wk, Wv, Wo):
    """Shard + lay out full inputs for the 8 cores."""
    Q = np.asarray(Q, dtype=np.float32)
    K = np.asarray(K, dtype=np.float32)
    V = np.asarray(V, dtype=np.float32)
    # pre-scale Wq by A_EXP: scores come out as A_EXP*s, so the DVE exp
    # path is a single add and the ScalarE path un-scales for free
    Wq = np.asarray(Wq, dtype=np.float32) * A_EXP
    Wk = np.asarray(Wk, dtype=np.float32)
    Wv = np.asarray(Wv, dtype=np.float32)
    Wo = np.asarray(Wo, dtype=np.float32)

    in_maps = []
    for c in range(NCORES):
        b, g = c // 2, c % 2
        rows = slice(g * F, (g + 1) * F)
        in_maps.append({
            "xq": np.ascontiguousarray(Q[b].T).astype(np.float16),
            "xk": np.ascontiguousarray(K[b].T).astype(np.float16),
            "xv": np.ascontiguousarray(V[b].T).astype(np.float16),
            "wq": np.ascontiguousarray(Wq[rows, :].T).astype(np.float16),
            "wk": np.ascontiguousarray(Wk[rows, :].T).astype(np.float16),
            "wv": np.ascontiguousarray(Wv[rows, :].T).astype(np.float16),
            "wo": np.ascontiguousarray(Wo[:, rows].T).astype(np.float16),
        })
    return in_maps


def combine(results, bo):
    """Sum per-core partials + bias -> full [B, S, E] output."""
    bo = np.asarray(bo, dtype=np.float32)
    y = np.zeros((B, S, E), dtype=np.float32)
    for c in range(NCORES):
        y[c // 2] += results[c]["y"]
    y += bo[None, None, :]
    return y


def kernel(Q, K, V, Wq, Wk, Wv, Wo, bo):
    from concourse.bass_utils import run_bass_kernel_spmd

    if "nc" not in _CACHE:
        _CACHE["nc"] = build_nc(reps=1)
    nc = _CACHE["nc"]
    in_maps = make_in_maps(Q, K, V, Wq, Wk, Wv, Wo)
    res = run_bass_kernel_spmd(nc, in_maps, core_ids=list(range(NCORES)))
    return combine(res.results, bo)
